# revision 16
# baseline (speedup 1.0000x reference)
"""Causal multi-head attention on 8 Trainium2 NeuronCores (bf16 pipeline).

Sharding: core c -> (batch b = c//2, head-group g = c%2 of 6 heads).
Each core computes its 6 heads' attention output contracted through its
slice of W_O; the two half-head partial outputs per batch are summed on
the host (no device collectives), and b_O is added on the host.

v2 design (all matmuls bf16 -> fp32 PSUM; FWL weight loads):
  - host supplies xT = x[b].T  [768, 2048] bf16 so no on-device transpose
  - QT/KT [384, 2048] = (wq|wk).T @ xT, bias added on DVE (tensor_scalar)
  - V [2048, 6*66] bf16 with a ones-column per head (66-stride keeps
    4B alignment) so the softmax denominator falls out of PV as row 64
  - scores: S^T tiles [128k, <=512q] = KT.T @ QT, two heads row-packed
    in the PE array (row groups 0-1 / 2-3, concurrent); causal TRIM:
    boundary k-tiles only compute q >= k0 (width 512-128r)
  - score tiles packed (whole k-tiles, greedy <=1024 cols) into a fused
    A/B PSUM staging tile [128, 2048] f32 (4 banks); ONE ACT exp call
    per generation covers both heads -> bf16 e in SBUF
  - causal 0/1 triangle mask post-exp on DVE (bf16 2x), 128 cols/bndry
  - PV: z^T[65, 512] += V_aug.T @ e per k-tile, fp32 PSUM accumulate
  - denom broadcast via K=1 ones matmul; 1/denom via DVE
    reciprocal_approx_fast; zhat = z * recip on DVE (bf16 out)
  - out[128q, 768] = sum_j zhat_pair.T @ wo_pair, bf16 out DMA, host
    sums the two head-group partials in fp32 and adds b_O
"""

import numpy as np

B = 4
S = 2048
D = 768
NH = 12
DH = 64
G = 2            # head groups (tensor-parallel)
HPG = NH // G    # heads per group = 6
NP = HPG // 2    # head pairs per group = 3
KT = D // 128    # 6 k-tiles of the d_model contraction
ST = S // 128    # 16 s-tiles
QC = S // 512    # 4 q-chunks
N_CORES = 8
VS = 66          # per-head stride in the V tile (64 V + 1 ones + 1 pad)


def _split_drain_waits(nc, mybir, max_waits=1):
    """This container's walrus only accepts one sync wait per instruction;
    hoist extra waits onto preceding single-wait NoOps on the same engine
    (engines execute in program order, so the waits still gate the inst)."""
    for f in nc.m.functions:
        for bb in f.blocks:
            newlist = []
            for ins in bb.instructions:
                si = ins.sync_info
                if si is not None and si.on_wait and len(si.on_wait) > max_waits:
                    waits = list(si.on_wait)
                    for i, w in enumerate(waits[:-max_waits]):
                        d = mybir.InstNoOp(name=f"{ins.name}-sw{i}", ins=[], outs=[])
                        d.engine = ins.engine
                        d.sync_info = mybir.SyncInfo(on_wait=[w], on_update=[])
                        newlist.append(d)
                    ins.sync_info = mybir.SyncInfo(
                        on_wait=list(waits[-max_waits:]), on_update=list(si.on_update)
                    )
                newlist.append(ins)
            try:
                bb.instructions = newlist
            except Exception:
                bb.instructions.clear()
                bb.instructions.extend(newlist)


def _chunk_gens(c):
    """Greedy-pack whole k-tiles of chunk c into generations of <=512
    score columns per head. Returns list of gens; each gen is a list of
    (t, off, w, qoff, boundary)."""
    kts = []
    for t in range(4 * c + 4):
        r = t - 4 * c
        if r < 0:
            kts.append((t, 512, 0, False))
        else:
            kts.append((t, 512 - 128 * r, 128 * r, True))
    gens, cur, width = [], [], 0
    for (t, w, qoff, bnd) in kts:
        if width + w > 512:
            gens.append(cur)
            cur, width = [], 0
        cur.append((t, width, w, qoff, bnd))
        width += w
    if cur:
        gens.append(cur)
    return gens


def build_program():
    import concourse.bass as bass
    import concourse.mybir as mybir
    import concourse.tile as tile

    f32 = mybir.dt.float32
    f32r = mybir.dt.float32r
    bf16 = mybir.dt.bfloat16
    EXP = mybir.ActivationFunctionType.Exp

    nc = bass.Bass("TRN2")
    xT = nc.dram_tensor("xT", [D, S], bf16, kind="ExternalInput")
    wq = nc.dram_tensor("wq", [D, HPG * DH], bf16, kind="ExternalInput")
    wk = nc.dram_tensor("wk", [D, HPG * DH], bf16, kind="ExternalInput")
    wv = nc.dram_tensor("wv", [D, HPG * DH], bf16, kind="ExternalInput")
    wo = nc.dram_tensor("wo", [HPG * DH, D], bf16, kind="ExternalInput")
    bq = nc.dram_tensor("bq", [HPG * DH], f32, kind="ExternalInput")
    bk = nc.dram_tensor("bk", [HPG * DH], f32, kind="ExternalInput")
    bv = nc.dram_tensor("bv", [1, HPG * DH], bf16, kind="ExternalInput")
    m01 = nc.dram_tensor("m01", [128, 128], bf16, kind="ExternalInput")
    ones_d = nc.dram_tensor("ones_d", [1, 128], bf16, kind="ExternalInput")
    ones2_d = nc.dram_tensor("ones2_d", [128, 128], f32, kind="ExternalInput")
    out = nc.dram_tensor("out", [S, D], bf16, kind="ExternalOutput")

    HD = HPG * DH  # 384

    from contextlib import ExitStack

    with tile.TileContext(nc) as tc:
        with ExitStack() as _ctx:
            _e = _ctx.enter_context
            _e(nc.allow_low_precision(reason="bf16 matmul pipeline"))
            wpool = _e(tc.tile_pool(name="weights", bufs=1))
            xtpool = _e(tc.tile_pool(name="xt", bufs=KT))
            qtpool = _e(tc.tile_pool(name="qt", bufs=NP * QC))
            ktpool = _e(tc.tile_pool(name="kt", bufs=NP * QC))
            vpool = _e(tc.tile_pool(name="v", bufs=ST))
            epool = _e(tc.tile_pool(name="e", bufs=3))
            zupool = _e(tc.tile_pool(name="zu", bufs=2))
            dnpool = _e(tc.tile_pool(name="dn", bufs=2))
            dnppool = _e(tc.tile_pool(name="dnp", bufs=2))
            zhpool = _e(tc.tile_pool(name="zh", bufs=2 * NP))
            opool = _e(tc.tile_pool(name="osb", bufs=3))
            stpool = _e(tc.tile_pool(name="st", bufs=1, space="PSUM"))
            pzpool = _e(tc.tile_pool(name="pz", bufs=2, space="PSUM"))
            miscpool = _e(tc.tile_pool(name="misc", bufs=4, space="PSUM"))

            # ---- small constants first (block nothing) ----
            ones_sb = wpool.tile([1, 128], bf16, tag="ones")
            nc.sync.dma_start(ones_sb[:], ones_d[:])
            ones2_sb = wpool.tile([128, 128], f32r, tag="ones2")
            nc.gpsimd.dma_start(ones2_sb[:], ones2_d[:].bitcast(f32r))
            bq_sb = wpool.tile([128, NP], f32, tag="bq")
            nc.sync.dma_start(bq_sb[:], bq[:].rearrange("(j p) -> p j", p=128))
            bk_sb = wpool.tile([128, NP], f32, tag="bk")
            nc.sync.dma_start(bk_sb[:], bk[:].rearrange("(j p) -> p j", p=128))
            bv_sb = wpool.tile([1, HD], bf16, tag="bv")
            nc.gpsimd.dma_start(bv_sb[:], bv[:])
            m01_sb = wpool.tile([128, 128], bf16, tag="m01")
            nc.gpsimd.dma_start(m01_sb[:], m01[:])

            # ---- PE warmup while input DMAs run (HAM un-throttle) ----
            wu = miscpool.tile([128, 128], f32, tag="misc", name="wu")
            for _ in range(20):
                nc.tensor.matmul(
                    wu[:], ones2_sb[:], ones2_sb[:], start=True, stop=True
                )

            # ---- weights and activations, split per k-tile block over two
            # DGE queues so the first projection matmuls start early ----
            wq_sb = wpool.tile([128, KT * HD], bf16, tag="wq")
            wk_sb = wpool.tile([128, KT * HD], bf16, tag="wk")
            wv_sb = wpool.tile([128, KT * HD], bf16, tag="wv")
            xt_sb = [xtpool.tile([128, S], bf16, tag="xt", name=f"xt{i}") for i in range(KT)]
            for a in range(KT):
                asl = slice(a * 128, (a + 1) * 128)
                nc.sync.dma_start(wq_sb[:, a * HD : (a + 1) * HD], wq[asl, :])
                nc.gpsimd.dma_start(wk_sb[:, a * HD : (a + 1) * HD], wk[asl, :])
            for c in range(QC):
                csl = slice(c * 512, (c + 1) * 512)
                for a in range(KT):
                    eng = nc.sync if a % 2 == 0 else nc.gpsimd
                    eng.dma_start(xt_sb[a][:, csl], xT[a * 128 : (a + 1) * 128, csl])
                if c == 0:
                    for a in range(KT):
                        eng = nc.sync if a % 2 == 1 else nc.gpsimd
                        eng.dma_start(
                            wv_sb[:, a * HD : (a + 1) * HD],
                            wv[a * 128 : (a + 1) * 128, :],
                        )
            wo_sb = wpool.tile([128, NP * D], bf16, tag="wo")
            nc.sync.dma_start(
                wo_sb[:].rearrange("p (j d) -> p j d", j=NP),
                wo[:].rearrange("(j p) d -> p j d", p=128),
            )

            # ---- persistent SBUF activation tiles ----
            qt_sb = [
                [qtpool.tile([128, 512], bf16, tag="qt", name=f"qt{i}_{cc}")
                 for cc in range(QC)]
                for i in range(NP)
            ]
            kt_sb = [
                [ktpool.tile([128, 512], bf16, tag="kt", name=f"kt{i}_{cc}")
                 for cc in range(QC)]
                for i in range(NP)
            ]
            v_sb = [vpool.tile([128, HPG * VS], bf16, tag="v", name=f"v{i}")
                    for i in range(ST)]

            def emit_proj_qk(c, j, which):
                csl = slice(c * 512, (c + 1) * 512)
                dst, w_sb, b_sb = (
                    (qt_sb[j][c], wq_sb, bq_sb)
                    if which == 0
                    else (kt_sb[j][c], wk_sb, bk_sb)
                )
                ps = miscpool.tile([128, 512], f32, tag="misc", name="ps")
                for a in range(KT):
                    nc.tensor.matmul(
                        ps[:],
                        w_sb[:, a * HD + j * 128 : a * HD + (j + 1) * 128],
                        xt_sb[a][:, csl],
                        start=(a == 0),
                        stop=(a == KT - 1),
                    )
                # PSUM->SBUF move + per-partition bias on DVE
                nc.vector.tensor_scalar_add(dst[:], ps[:], b_sb[:, j : j + 1])

            def emit_proj_v(st):
                vt = v_sb[st]
                pv = miscpool.tile([128, HD + HPG], f32, tag="misc", name="pv")
                for a in range(KT):
                    nc.tensor.matmul(
                        pv[:, 0:HD],
                        xt_sb[a][:, st * 128 : (st + 1) * 128],
                        wv_sb[:, a * HD : (a + 1) * HD],
                        start=(a == 0),
                        stop=False,
                    )
                nc.tensor.matmul(
                    pv[:, 0:HD], ones_sb[:, :], bv_sb[:, :],
                    start=False, stop=True,
                )
                # ones columns for the softmax denominator (z row 64)
                nc.tensor.matmul(
                    pv[:, HD : HD + HPG],
                    ones_sb[:, :], ones_sb[:, 0:HPG],
                    start=True, stop=True,
                )
                vtv = vt[:].rearrange("p (n c) -> p n c", n=HPG)
                nc.vector.tensor_copy(
                    vtv[:, :, 0:DH],
                    pv[:, 0:HD].rearrange("p (n c) -> p n c", n=HPG),
                )
                nc.vector.tensor_copy(
                    vtv[:, :, DH : DH + 1],
                    pv[:, HD : HD + HPG].rearrange("p (n c) -> p n c", n=HPG),
                )

            def proj_units(c):
                for j in range(NP):
                    yield lambda j=j: emit_proj_qk(c, j, 0)
                    yield lambda j=j: emit_proj_qk(c, j, 1)
                for st in range(4 * c, 4 * c + 4):
                    yield lambda st=st: emit_proj_v(st)

            # chunk 0 projections up front
            for u in proj_units(0):
                u()

            for c in range(QC):
                # filler: next chunk's projections, interleaved between
                # attention generations so the in-order PE queue always has
                # independent matmuls behind the exp-gated score matmuls.
                filler = iter(proj_units(c + 1)) if c + 1 < QC else iter(())

                # ---- attention for this q-chunk ----
                gens = _chunk_gens(c)
                zh = [None] * NP
                for j in range(NP):
                    pzA = pzpool.tile([65, 512], f32, tag="pz", name="pzA")
                    pzB = pzpool.tile([65, 512], f32, tag="pz", name="pzB")
                    genctr = 0
                    for gen in gens:
                        genctr += 1
                        gw = gen[-1][1] + gen[-1][2]  # packed width
                        stg = stpool.tile([128, 1024], f32, tag="st", name="stg")
                        for (t, off, w, qoff, bnd) in gen:
                            kc, ko = t // 4, (t % 4) * 128
                            nc.tensor.matmul(
                                stg[:, off : off + w],
                                kt_sb[j][kc][0:64, ko : ko + 128],
                                qt_sb[j][c][0:64, qoff : qoff + w],
                                start=True, stop=True,
                            )
                            nc.tensor.matmul(
                                stg[:, 512 + off : 512 + off + w],
                                kt_sb[j][kc][64:128, ko : ko + 128],
                                qt_sb[j][c][64:128, qoff : qoff + w],
                                start=True, stop=True,
                            )
                        # one exp over both heads' packed region
                        et = epool.tile([128, 1024], bf16, tag="e", name="et")
                        if gw == 512:
                            nc.scalar.activation(et[:], stg[:], EXP, scale=0.125)
                        else:
                            nc.scalar.activation(
                                et[:, 0:gw], stg[:, 0:gw], EXP, scale=0.125
                            )
                            nc.scalar.activation(
                                et[:, 512 : 512 + gw],
                                stg[:, 512 : 512 + gw],
                                EXP,
                                scale=0.125,
                            )
                        # causal 0/1 triangle on boundary tiles (post-exp)
                        for (t, off, w, qoff, bnd) in gen:
                            if bnd:
                                for half in range(2):
                                    ho = 512 * half + off
                                    nc.vector.tensor_mul(
                                        et[:, ho : ho + 128],
                                        et[:, ho : ho + 128],
                                        m01_sb[:],
                                    )
                        # PV accumulate
                        for (t, off, w, qoff, bnd) in gen:
                            last = t == 4 * c + 3
                            nc.tensor.matmul(
                                pzA[:, qoff : qoff + w],
                                v_sb[t][:, (2 * j) * VS : (2 * j) * VS + 65],
                                et[:, off : off + w],
                                start=(t == 0), stop=last,
                            )
                            nc.tensor.matmul(
                                pzB[:, qoff : qoff + w],
                                v_sb[t][:, (2 * j + 1) * VS : (2 * j + 1) * VS + 65],
                                et[:, 512 + off : 512 + off + w],
                                start=(t == 0), stop=last,
                            )
                        if genctr % 3 == 0:
                            u = next(filler, None)
                            if u is not None:
                                u()
                    # ---- z / denom staging ----
                    # denominators are row-shaped [1, 512]; a straight DVE
                    # reciprocal on rows is lane-starved (512 elems on one
                    # lane). Spread them over partitions with a small DMA,
                    # reciprocal [128, 8], spread back, then broadcast the
                    # recip row to all 128 partitions with a K=1 matmul.
                    zu = zupool.tile([128, 512], f32, tag="zu", name="zu")
                    dn = dnpool.tile([65, 1024], f32, tag="dn", name="dn")
                    nc.vector.tensor_copy(zu[0:64, :], pzA[0:64, :])
                    nc.vector.tensor_copy(zu[64:128, :], pzB[0:64, :])
                    nc.vector.tensor_copy(dn[64:65, 0:512], pzA[64:65, :])
                    nc.vector.tensor_copy(dn[64:65, 512:1024], pzB[64:65, :])
                    dnp = dnppool.tile([128, 8], f32, tag="dnp", name="dnp")
                    nc.gpsimd.dma_start(dnp[:], dn[64:65, :])
                    rp = dnppool.tile([128, 8], f32r, tag="rp", name="rp")
                    nc.vector.reciprocal(rp[:], dnp[:])
                    rrow = dnpool.tile([65, 1024], f32r, tag="rrow", name="rrow")
                    nc.gpsimd.dma_start(rrow[64:65, :], rp[:])
                    bcpA = miscpool.tile([128, 512], f32, tag="misc", name="bcpA")
                    nc.tensor.matmul(
                        bcpA[:], ones2_sb[64:65, 0:128], rrow[64:65, 0:512],
                        start=True, stop=True,
                    )
                    bcpB = miscpool.tile([128, 512], f32, tag="misc", name="bcpB")
                    nc.tensor.matmul(
                        bcpB[:], ones2_sb[64:65, 0:128], rrow[64:65, 512:1024],
                        start=True, stop=True,
                    )
                    zht = zhpool.tile([128, 512], bf16, tag="zh", name=f"zh{j}")
                    nc.vector.tensor_mul(zht[0:64, :], zu[0:64, :], bcpA[0:64, :])
                    nc.vector.tensor_mul(zht[64:128, :], zu[64:128, :], bcpB[64:128, :])
                    zh[j] = zht
                    for u in (next(filler, None),):
                        if u is not None:
                            u()

                # drain any remaining filler before W_O
                for u in filler:
                    u()

                # ---- W_O contraction for this q-chunk ----
                for qs in range(4):
                    row = c * 512 + qs * 128
                    for half in range(2):
                        po = miscpool.tile([128, 384], f32, tag="misc", name="po")
                        for j in range(NP):
                            nc.tensor.matmul(
                                po[:],
                                zh[j][:, qs * 128 : (qs + 1) * 128],
                                wo_sb[:, j * D + half * 384 : j * D + (half + 1) * 384],
                                start=(j == 0), stop=(j == NP - 1),
                            )
                        osb = opool.tile([128, 384], bf16, tag="osb", name="osb")
                        nc.vector.tensor_copy(osb[:], po[:])
                        nc.sync.dma_start(
                            out[row : row + 128, half * 384 : (half + 1) * 384],
                            osb[:],
                        )

    _split_drain_waits(nc, mybir)
    return nc


_nc_cache = None


def kernel(normalized_resid_pre, W_Q, W_K, W_V, W_O, b_Q, b_K, b_V, b_O):
    import ml_dtypes
    from concourse.bass_utils import run_bass_kernel_spmd

    global _nc_cache
    if _nc_cache is None:
        _nc_cache = build_program()
    nc = _nc_cache

    bf16 = ml_dtypes.bfloat16
    x = np.asarray(normalized_resid_pre, np.float32)

    # multiplicative causal mask for the diagonal 128x128 block:
    # keep (1.0) where k_local <= q_local, else 0.
    p = np.arange(128)[:, None]
    u = np.arange(128)[None, :]
    m01 = np.where(p <= u, 1.0, 0.0).astype(bf16)

    in_maps = []
    for c in range(N_CORES):
        b, g = c // G, c % G
        hs = slice(g * HPG, (g + 1) * HPG)
        in_maps.append(
            {
                "xT": np.ascontiguousarray(x[b].T).astype(bf16),
                "wq": np.ascontiguousarray(
                    np.asarray(W_Q)[hs].transpose(1, 0, 2).reshape(D, HPG * DH)
                ).astype(bf16),
                "wk": np.ascontiguousarray(
                    np.asarray(W_K)[hs].transpose(1, 0, 2).reshape(D, HPG * DH)
                ).astype(bf16),
                "wv": np.ascontiguousarray(
                    np.asarray(W_V)[hs].transpose(1, 0, 2).reshape(D, HPG * DH)
                ).astype(bf16),
                "wo": np.ascontiguousarray(
                    np.asarray(W_O)[hs].reshape(HPG * DH, D)
                ).astype(bf16),
                "bq": np.ascontiguousarray(
                    np.asarray(b_Q, np.float32)[hs].reshape(-1)
                ),
                "bk": np.ascontiguousarray(
                    np.asarray(b_K, np.float32)[hs].reshape(-1)
                ),
                "bv": np.ascontiguousarray(
                    np.asarray(b_V)[hs].reshape(1, -1)
                ).astype(bf16),
                "m01": m01,
                "ones_d": np.ones((1, 128), bf16),
                "ones2_d": np.ones((128, 128), np.float32),
            }
        )

    res = run_bass_kernel_spmd(nc, in_maps, core_ids=list(range(N_CORES)))
    out = np.zeros((B, S, D), np.float32)
    for c in range(N_CORES):
        out[c // G] += np.asarray(res.results[c]["out"], np.float32)
    out += np.asarray(b_O, np.float32)
    return out


# revision 18
# speedup vs baseline: 1.4642x; 1.4642x over previous
"""Causal multi-head attention on 8 Trainium2 NeuronCores (bf16 pipeline).

Sharding: core c -> (batch b = c//2, head-group g = c%2 of 6 heads).
Each core computes its 6 heads' attention output contracted through its
slice of W_O; the two half-head partial outputs per batch are summed on
the host (no device collectives), and b_O is added on the host.

v2 design (all matmuls bf16 -> fp32 PSUM; FWL weight loads):
  - host supplies xT = x[b].T  [768, 2048] bf16 so no on-device transpose
  - QT/KT [384, 2048] = (wq|wk).T @ xT, bias added on DVE (tensor_scalar)
  - V [2048, 6*66] bf16 with a ones-column per head (66-stride keeps
    4B alignment) so the softmax denominator falls out of PV as row 64
  - scores: S^T tiles [128k, <=512q] = KT.T @ QT, two heads row-packed
    in the PE array (row groups 0-1 / 2-3, concurrent); causal TRIM:
    boundary k-tiles only compute q >= k0 (width 512-128r)
  - score tiles packed (whole k-tiles, greedy <=1024 cols) into a fused
    A/B PSUM staging tile [128, 2048] f32 (4 banks); ONE ACT exp call
    per generation covers both heads -> bf16 e in SBUF
  - causal 0/1 triangle mask post-exp on DVE (bf16 2x), 128 cols/bndry
  - PV: z^T[65, 512] += V_aug.T @ e per k-tile, fp32 PSUM accumulate
  - denom broadcast via K=1 ones matmul; 1/denom via DVE
    reciprocal_approx_fast; zhat = z * recip on DVE (bf16 out)
  - out[128q, 768] = sum_j zhat_pair.T @ wo_pair, bf16 out DMA, host
    sums the two head-group partials in fp32 and adds b_O
"""

import numpy as np

B = 4
S = 2048
D = 768
NH = 12
DH = 64
G = 2            # head groups (tensor-parallel)
HPG = NH // G    # heads per group = 6
NP = HPG // 2    # head pairs per group = 3
KT = D // 128    # 6 k-tiles of the d_model contraction
ST = S // 128    # 16 s-tiles
QC = S // 512    # 4 q-chunks
N_CORES = 8
VS = 66          # per-head stride in the V tile (64 V + 1 ones + 1 pad)


def _split_drain_waits(nc, mybir, max_waits=1):
    """This container's walrus only accepts one sync wait per instruction;
    hoist extra waits onto preceding single-wait NoOps on the same engine
    (engines execute in program order, so the waits still gate the inst)."""
    for f in nc.m.functions:
        for bb in f.blocks:
            newlist = []
            for ins in bb.instructions:
                si = ins.sync_info
                if si is not None and si.on_wait and len(si.on_wait) > max_waits:
                    waits = list(si.on_wait)
                    for i, w in enumerate(waits[:-max_waits]):
                        d = mybir.InstNoOp(name=f"{ins.name}-sw{i}", ins=[], outs=[])
                        d.engine = ins.engine
                        d.sync_info = mybir.SyncInfo(on_wait=[w], on_update=[])
                        newlist.append(d)
                    ins.sync_info = mybir.SyncInfo(
                        on_wait=list(waits[-max_waits:]), on_update=list(si.on_update)
                    )
                newlist.append(ins)
            try:
                bb.instructions = newlist
            except Exception:
                bb.instructions.clear()
                bb.instructions.extend(newlist)


def _chunk_gens(c):
    """Greedy-pack whole k-tiles of chunk c into generations of <=1024
    score columns per head. Returns list of gens; each gen is a list of
    (t, off, w, qoff, boundary)."""
    kts = []
    for t in range(4 * c + 4):
        r = t - 4 * c
        if r < 0:
            kts.append((t, 512, 0, False))
        else:
            kts.append((t, 512 - 128 * r, 128 * r, True))
    gens, cur, width = [], [], 0
    for (t, w, qoff, bnd) in kts:
        if width + w > 1024:
            gens.append(cur)
            cur, width = [], 0
        cur.append((t, width, w, qoff, bnd))
        width += w
    if cur:
        gens.append(cur)
    return gens


def build_program():
    import concourse.bass as bass
    import concourse.mybir as mybir
    import concourse.tile as tile

    f32 = mybir.dt.float32
    f32r = mybir.dt.float32r
    bf16 = mybir.dt.bfloat16
    EXP = mybir.ActivationFunctionType.Exp

    nc = bass.Bass("TRN2")
    xT = nc.dram_tensor("xT", [D, S], bf16, kind="ExternalInput")
    wq = nc.dram_tensor("wq", [D, HPG * DH], bf16, kind="ExternalInput")
    wk = nc.dram_tensor("wk", [D, HPG * DH], bf16, kind="ExternalInput")
    wv = nc.dram_tensor("wv", [D, HPG * DH], bf16, kind="ExternalInput")
    wo = nc.dram_tensor("wo", [HPG * DH, D], bf16, kind="ExternalInput")
    bq = nc.dram_tensor("bq", [HPG * DH], f32, kind="ExternalInput")
    bk = nc.dram_tensor("bk", [HPG * DH], f32, kind="ExternalInput")
    bv = nc.dram_tensor("bv", [1, HPG * DH], bf16, kind="ExternalInput")
    m01 = nc.dram_tensor("m01", [128, 128], bf16, kind="ExternalInput")
    ones_d = nc.dram_tensor("ones_d", [1, 128], bf16, kind="ExternalInput")
    ones2_d = nc.dram_tensor("ones2_d", [128, 128], f32, kind="ExternalInput")
    out = nc.dram_tensor("out", [S, D], bf16, kind="ExternalOutput")

    HD = HPG * DH  # 384

    from contextlib import ExitStack

    with tile.TileContext(nc) as tc:
        with ExitStack() as _ctx:
            _e = _ctx.enter_context
            _e(nc.allow_low_precision(reason="bf16 matmul pipeline"))
            wpool = _e(tc.tile_pool(name="weights", bufs=1))
            xtpool = _e(tc.tile_pool(name="xt", bufs=KT))
            qtpool = _e(tc.tile_pool(name="qt", bufs=NP * QC))
            ktpool = _e(tc.tile_pool(name="kt", bufs=NP * QC))
            vpool = _e(tc.tile_pool(name="v", bufs=ST))
            epool = _e(tc.tile_pool(name="e", bufs=3))
            zupool = _e(tc.tile_pool(name="zu", bufs=2))
            dnpool = _e(tc.tile_pool(name="dn", bufs=2))
            dnppool = _e(tc.tile_pool(name="dnp", bufs=2))
            zhpool = _e(tc.tile_pool(name="zh", bufs=2 * NP))
            opool = _e(tc.tile_pool(name="osb", bufs=3))
            stpool = _e(tc.tile_pool(name="st", bufs=1, space="PSUM"))
            pzpool = _e(tc.tile_pool(name="pz", bufs=2, space="PSUM"))
            miscpool = _e(tc.tile_pool(name="misc", bufs=2, space="PSUM"))

            # ---- small constants first (block nothing) ----
            ones_sb = wpool.tile([1, 128], bf16, tag="ones")
            nc.sync.dma_start(ones_sb[:], ones_d[:])
            ones2_sb = wpool.tile([128, 128], f32r, tag="ones2")
            nc.gpsimd.dma_start(ones2_sb[:], ones2_d[:].bitcast(f32r))
            bq_sb = wpool.tile([128, NP], f32, tag="bq")
            nc.sync.dma_start(bq_sb[:], bq[:].rearrange("(j p) -> p j", p=128))
            bk_sb = wpool.tile([128, NP], f32, tag="bk")
            nc.sync.dma_start(bk_sb[:], bk[:].rearrange("(j p) -> p j", p=128))
            bv_sb = wpool.tile([1, HD], bf16, tag="bv")
            nc.gpsimd.dma_start(bv_sb[:], bv[:])
            m01_sb = wpool.tile([128, 128], bf16, tag="m01")
            nc.gpsimd.dma_start(m01_sb[:], m01[:])

            # ---- PE warmup while input DMAs run (HAM un-throttle) ----
            wu = miscpool.tile([128, 128], f32, tag="misc", name="wu")
            for _ in range(20):
                nc.tensor.matmul(
                    wu[:], ones2_sb[:], ones2_sb[:], start=True, stop=True
                )

            # ---- weights and activations, split per k-tile block over two
            # DGE queues so the first projection matmuls start early ----
            wq_sb = wpool.tile([128, KT * HD], bf16, tag="wq")
            wk_sb = wpool.tile([128, KT * HD], bf16, tag="wk")
            wv_sb = wpool.tile([128, KT * HD], bf16, tag="wv")
            xt_sb = [xtpool.tile([128, S], bf16, tag="xt", name=f"xt{i}") for i in range(KT)]
            for a in range(KT):
                asl = slice(a * 128, (a + 1) * 128)
                nc.sync.dma_start(wq_sb[:, a * HD : (a + 1) * HD], wq[asl, :])
                nc.gpsimd.dma_start(wk_sb[:, a * HD : (a + 1) * HD], wk[asl, :])
            for c in range(QC):
                csl = slice(c * 512, (c + 1) * 512)
                for a in range(KT):
                    eng = nc.sync if a % 2 == 0 else nc.gpsimd
                    eng.dma_start(xt_sb[a][:, csl], xT[a * 128 : (a + 1) * 128, csl])
                if c == 0:
                    for a in range(KT):
                        eng = nc.sync if a % 2 == 1 else nc.gpsimd
                        eng.dma_start(
                            wv_sb[:, a * HD : (a + 1) * HD],
                            wv[a * 128 : (a + 1) * 128, :],
                        )
            wo_sb = wpool.tile([128, NP * D], bf16, tag="wo")
            nc.sync.dma_start(
                wo_sb[:].rearrange("p (j d) -> p j d", j=NP),
                wo[:].rearrange("(j p) d -> p j d", p=128),
            )

            # ---- persistent SBUF activation tiles ----
            qt_sb = [
                [qtpool.tile([128, 512], bf16, tag="qt", name=f"qt{i}_{cc}")
                 for cc in range(QC)]
                for i in range(NP)
            ]
            kt_sb = [
                [ktpool.tile([128, 512], bf16, tag="kt", name=f"kt{i}_{cc}")
                 for cc in range(QC)]
                for i in range(NP)
            ]
            v_sb = [vpool.tile([128, HPG * VS], bf16, tag="v", name=f"v{i}")
                    for i in range(ST)]

            def emit_proj_qk(c, j, which):
                csl = slice(c * 512, (c + 1) * 512)
                dst, w_sb, b_sb = (
                    (qt_sb[j][c], wq_sb, bq_sb)
                    if which == 0
                    else (kt_sb[j][c], wk_sb, bk_sb)
                )
                ps = miscpool.tile([128, 512], f32, tag="misc", name="ps")
                for a in range(KT):
                    nc.tensor.matmul(
                        ps[:],
                        w_sb[:, a * HD + j * 128 : a * HD + (j + 1) * 128],
                        xt_sb[a][:, csl],
                        start=(a == 0),
                        stop=(a == KT - 1),
                    )
                # PSUM->SBUF move + per-partition bias on DVE
                nc.vector.tensor_scalar_add(dst[:], ps[:], b_sb[:, j : j + 1])

            def emit_proj_v(st):
                vt = v_sb[st]
                pv = miscpool.tile([128, HD + HPG], f32, tag="misc", name="pv")
                for a in range(KT):
                    nc.tensor.matmul(
                        pv[:, 0:HD],
                        xt_sb[a][:, st * 128 : (st + 1) * 128],
                        wv_sb[:, a * HD : (a + 1) * HD],
                        start=(a == 0),
                        stop=False,
                    )
                nc.tensor.matmul(
                    pv[:, 0:HD], ones_sb[:, :], bv_sb[:, :],
                    start=False, stop=True,
                )
                # ones columns for the softmax denominator (z row 64)
                nc.tensor.matmul(
                    pv[:, HD : HD + HPG],
                    ones_sb[:, :], ones_sb[:, 0:HPG],
                    start=True, stop=True,
                )
                vtv = vt[:].rearrange("p (n c) -> p n c", n=HPG)
                nc.vector.tensor_copy(
                    vtv[:, :, 0:DH],
                    pv[:, 0:HD].rearrange("p (n c) -> p n c", n=HPG),
                )
                nc.vector.tensor_copy(
                    vtv[:, :, DH : DH + 1],
                    pv[:, HD : HD + HPG].rearrange("p (n c) -> p n c", n=HPG),
                )

            def proj_units(c):
                for j in range(NP):
                    yield lambda j=j: emit_proj_qk(c, j, 0)
                    yield lambda j=j: emit_proj_qk(c, j, 1)
                for st in range(4 * c, 4 * c + 4):
                    yield lambda st=st: emit_proj_v(st)

            # chunk 0 projections up front
            for u in proj_units(0):
                u()

            for c in range(QC):
                # filler: next chunk's projections, interleaved between
                # attention generations so the in-order PE queue always has
                # independent matmuls behind the exp-gated score matmuls.
                filler = iter(proj_units(c + 1)) if c + 1 < QC else iter(())

                # ---- attention for this q-chunk ----
                gens = _chunk_gens(c)
                zh = [None] * NP
                for j in range(NP):
                    pzA = pzpool.tile([65, 512], f32, tag="pz", name="pzA")
                    pzB = pzpool.tile([65, 512], f32, tag="pz", name="pzB")
                    def emit_pv(gen, et):
                        for (t, off, w, qoff, bnd) in gen:
                            last = t == 4 * c + 3
                            nc.tensor.matmul(
                                pzA[:, qoff : qoff + w],
                                v_sb[t][:, (2 * j) * VS : (2 * j) * VS + 65],
                                et[:, off : off + w],
                                start=(t == 0), stop=last,
                            )
                            nc.tensor.matmul(
                                pzB[:, qoff : qoff + w],
                                v_sb[t][:, (2 * j + 1) * VS : (2 * j + 1) * VS + 65],
                                et[:, 1024 + off : 1024 + off + w],
                                start=(t == 0), stop=last,
                            )

                    # software pipeline: PV trails scores/exp by one gen so
                    # the in-order PE queue never waits on the ACT exp.
                    prev = None
                    for gen in gens:
                        gw = gen[-1][1] + gen[-1][2]  # packed width
                        stg = stpool.tile([128, 2048], f32, tag="st", name="stg")
                        for (t, off, w, qoff, bnd) in gen:
                            kc, ko = t // 4, (t % 4) * 128
                            nc.tensor.matmul(
                                stg[:, off : off + w],
                                kt_sb[j][kc][0:64, ko : ko + 128],
                                qt_sb[j][c][0:64, qoff : qoff + w],
                                start=True, stop=True,
                            )
                            nc.tensor.matmul(
                                stg[:, 1024 + off : 1024 + off + w],
                                kt_sb[j][kc][64:128, ko : ko + 128],
                                qt_sb[j][c][64:128, qoff : qoff + w],
                                start=True, stop=True,
                            )
                        # one exp over both heads' packed region
                        et = epool.tile([128, 2048], bf16, tag="e", name="et")
                        if gw == 1024:
                            nc.scalar.activation(et[:], stg[:], EXP, scale=0.125)
                        else:
                            nc.scalar.activation(
                                et[:, 0:gw], stg[:, 0:gw], EXP, scale=0.125
                            )
                            nc.scalar.activation(
                                et[:, 1024 : 1024 + gw],
                                stg[:, 1024 : 1024 + gw],
                                EXP,
                                scale=0.125,
                            )
                        # causal 0/1 triangle on boundary tiles (post-exp)
                        for (t, off, w, qoff, bnd) in gen:
                            if bnd:
                                for half in range(2):
                                    ho = 1024 * half + off
                                    nc.vector.tensor_mul(
                                        et[:, ho : ho + 128],
                                        et[:, ho : ho + 128],
                                        m01_sb[:],
                                    )
                        if prev is not None:
                            emit_pv(*prev)
                        prev = (gen, et)
                        for u in (next(filler, None),):
                            if u is not None:
                                u()
                    if prev is not None:
                        emit_pv(*prev)
                    # ---- z / denom staging ----
                    # denominators are row-shaped [1, 512]; a straight DVE
                    # reciprocal on rows is lane-starved (512 elems on one
                    # lane). Spread them over partitions with a small DMA,
                    # reciprocal [128, 8], spread back, then broadcast the
                    # recip row to all 128 partitions with a K=1 matmul.
                    zu = zupool.tile([128, 512], f32, tag="zu", name="zu")
                    dn = dnpool.tile([65, 1024], f32, tag="dn", name="dn")
                    nc.vector.tensor_copy(zu[0:64, :], pzA[0:64, :])
                    nc.vector.tensor_copy(zu[64:128, :], pzB[0:64, :])
                    nc.vector.tensor_copy(dn[64:65, 0:512], pzA[64:65, :])
                    nc.vector.tensor_copy(dn[64:65, 512:1024], pzB[64:65, :])
                    dnp = dnppool.tile([128, 8], f32, tag="dnp", name="dnp")
                    nc.gpsimd.dma_start(dnp[:], dn[64:65, :])
                    rp = dnppool.tile([128, 8], f32r, tag="rp", name="rp")
                    nc.vector.reciprocal(rp[:], dnp[:])
                    rrow = dnpool.tile([65, 1024], f32r, tag="rrow", name="rrow")
                    nc.gpsimd.dma_start(rrow[64:65, :], rp[:])
                    bcpA = miscpool.tile([128, 512], f32, tag="misc", name="bcpA")
                    nc.tensor.matmul(
                        bcpA[:], ones2_sb[64:65, 0:128], rrow[64:65, 0:512],
                        start=True, stop=True,
                    )
                    bcpB = miscpool.tile([128, 512], f32, tag="misc", name="bcpB")
                    nc.tensor.matmul(
                        bcpB[:], ones2_sb[64:65, 0:128], rrow[64:65, 512:1024],
                        start=True, stop=True,
                    )
                    zht = zhpool.tile([128, 512], bf16, tag="zh", name=f"zh{j}")
                    nc.vector.tensor_mul(zht[0:64, :], zu[0:64, :], bcpA[0:64, :])
                    nc.vector.tensor_mul(zht[64:128, :], zu[64:128, :], bcpB[64:128, :])
                    zh[j] = zht
                    for u in (next(filler, None),):
                        if u is not None:
                            u()

                # drain any remaining filler before W_O
                for u in filler:
                    u()

                # ---- W_O contraction for this q-chunk ----
                for qs in range(4):
                    row = c * 512 + qs * 128
                    for half in range(2):
                        po = miscpool.tile([128, 384], f32, tag="misc", name="po")
                        for j in range(NP):
                            nc.tensor.matmul(
                                po[:],
                                zh[j][:, qs * 128 : (qs + 1) * 128],
                                wo_sb[:, j * D + half * 384 : j * D + (half + 1) * 384],
                                start=(j == 0), stop=(j == NP - 1),
                            )
                        osb = opool.tile([128, 384], bf16, tag="osb", name="osb")
                        nc.vector.tensor_copy(osb[:], po[:])
                        nc.sync.dma_start(
                            out[row : row + 128, half * 384 : (half + 1) * 384],
                            osb[:],
                        )

    _split_drain_waits(nc, mybir)
    return nc


_nc_cache = None


def kernel(normalized_resid_pre, W_Q, W_K, W_V, W_O, b_Q, b_K, b_V, b_O):
    import ml_dtypes
    from concourse.bass_utils import run_bass_kernel_spmd

    global _nc_cache
    if _nc_cache is None:
        _nc_cache = build_program()
    nc = _nc_cache

    bf16 = ml_dtypes.bfloat16
    x = np.asarray(normalized_resid_pre, np.float32)

    # multiplicative causal mask for the diagonal 128x128 block:
    # keep (1.0) where k_local <= q_local, else 0.
    p = np.arange(128)[:, None]
    u = np.arange(128)[None, :]
    m01 = np.where(p <= u, 1.0, 0.0).astype(bf16)

    in_maps = []
    for c in range(N_CORES):
        b, g = c // G, c % G
        hs = slice(g * HPG, (g + 1) * HPG)
        in_maps.append(
            {
                "xT": np.ascontiguousarray(x[b].T).astype(bf16),
                "wq": np.ascontiguousarray(
                    np.asarray(W_Q)[hs].transpose(1, 0, 2).reshape(D, HPG * DH)
                ).astype(bf16),
                "wk": np.ascontiguousarray(
                    np.asarray(W_K)[hs].transpose(1, 0, 2).reshape(D, HPG * DH)
                ).astype(bf16),
                "wv": np.ascontiguousarray(
                    np.asarray(W_V)[hs].transpose(1, 0, 2).reshape(D, HPG * DH)
                ).astype(bf16),
                "wo": np.ascontiguousarray(
                    np.asarray(W_O)[hs].reshape(HPG * DH, D)
                ).astype(bf16),
                "bq": np.ascontiguousarray(
                    np.asarray(b_Q, np.float32)[hs].reshape(-1)
                ),
                "bk": np.ascontiguousarray(
                    np.asarray(b_K, np.float32)[hs].reshape(-1)
                ),
                "bv": np.ascontiguousarray(
                    np.asarray(b_V)[hs].reshape(1, -1)
                ).astype(bf16),
                "m01": m01,
                "ones_d": np.ones((1, 128), bf16),
                "ones2_d": np.ones((128, 128), np.float32),
            }
        )

    res = run_bass_kernel_spmd(nc, in_maps, core_ids=list(range(N_CORES)))
    out = np.zeros((B, S, D), np.float32)
    for c in range(N_CORES):
        out[c // G] += np.asarray(res.results[c]["out"], np.float32)
    out += np.asarray(b_O, np.float32)
    return out


# revision 19
# speedup vs baseline: 1.7846x; 1.2188x over previous
"""Causal multi-head attention on 8 Trainium2 NeuronCores (bf16 pipeline).

Sharding: core c -> (batch b = c//2, head-group g = c%2 of 6 heads).
Each core computes its 6 heads' attention output contracted through its
slice of W_O; the two half-head partial outputs per batch are summed on
the host (no device collectives), and b_O is added on the host.

v2 design (all matmuls bf16 -> fp32 PSUM; FWL weight loads):
  - host supplies xT = x[b].T  [768, 2048] bf16 so no on-device transpose
  - QT/KT [384, 2048] = (wq|wk).T @ xT, bias added on DVE (tensor_scalar)
  - V [2048, 6*66] bf16 with a ones-column per head (66-stride keeps
    4B alignment) so the softmax denominator falls out of PV as row 64
  - scores: S^T tiles [128k, <=512q] = KT.T @ QT, two heads row-packed
    in the PE array (row groups 0-1 / 2-3, concurrent); causal TRIM:
    boundary k-tiles only compute q >= k0 (width 512-128r)
  - score tiles packed (whole k-tiles, greedy <=1024 cols) into a fused
    A/B PSUM staging tile [128, 2048] f32 (4 banks); ONE ACT exp call
    per generation covers both heads -> bf16 e in SBUF
  - causal 0/1 triangle mask post-exp on DVE (bf16 2x), 128 cols/bndry
  - PV: z^T[65, 512] += V_aug.T @ e per k-tile, fp32 PSUM accumulate
  - denom broadcast via K=1 ones matmul; 1/denom via DVE
    reciprocal_approx_fast; zhat = z * recip on DVE (bf16 out)
  - out[128q, 768] = sum_j zhat_pair.T @ wo_pair, bf16 out DMA, host
    sums the two head-group partials in fp32 and adds b_O
"""

import numpy as np

B = 4
S = 2048
D = 768
NH = 12
DH = 64
G = 2            # head groups (tensor-parallel)
HPG = NH // G    # heads per group = 6
NP = HPG // 2    # head pairs per group = 3
KT = D // 128    # 6 k-tiles of the d_model contraction
ST = S // 128    # 16 s-tiles
QC = S // 512    # 4 q-chunks
N_CORES = 8
VS = 66          # per-head stride in the V tile (64 V + 1 ones + 1 pad)


def _split_drain_waits(nc, mybir, max_waits=1):
    """This container's walrus only accepts one sync wait per instruction;
    hoist extra waits onto preceding single-wait NoOps on the same engine
    (engines execute in program order, so the waits still gate the inst)."""
    for f in nc.m.functions:
        for bb in f.blocks:
            newlist = []
            for ins in bb.instructions:
                si = ins.sync_info
                if si is not None and si.on_wait and len(si.on_wait) > max_waits:
                    waits = list(si.on_wait)
                    for i, w in enumerate(waits[:-max_waits]):
                        d = mybir.InstNoOp(name=f"{ins.name}-sw{i}", ins=[], outs=[])
                        d.engine = ins.engine
                        d.sync_info = mybir.SyncInfo(on_wait=[w], on_update=[])
                        newlist.append(d)
                    ins.sync_info = mybir.SyncInfo(
                        on_wait=list(waits[-max_waits:]), on_update=list(si.on_update)
                    )
                newlist.append(ins)
            try:
                bb.instructions = newlist
            except Exception:
                bb.instructions.clear()
                bb.instructions.extend(newlist)


def _chunk_gens(c):
    """Greedy-pack whole k-tiles of chunk c into generations of <=1024
    score columns per head. Returns list of gens; each gen is a list of
    (t, off, w, qoff, boundary)."""
    kts = []
    for t in range(4 * c + 4):
        r = t - 4 * c
        if r < 0:
            kts.append((t, 512, 0, False))
        else:
            kts.append((t, 512 - 128 * r, 128 * r, True))
    gens, cur, width = [], [], 0
    for (t, w, qoff, bnd) in kts:
        if width + w > 1024:
            gens.append(cur)
            cur, width = [], 0
        cur.append((t, width, w, qoff, bnd))
        width += w
    if cur:
        gens.append(cur)
    return gens


def build_program():
    import concourse.bass as bass
    import concourse.mybir as mybir
    import concourse.tile as tile

    f32 = mybir.dt.float32
    f32r = mybir.dt.float32r
    bf16 = mybir.dt.bfloat16
    EXP = mybir.ActivationFunctionType.Exp

    nc = bass.Bass("TRN2")
    xT = nc.dram_tensor("xT", [D, S], bf16, kind="ExternalInput")
    wq = nc.dram_tensor("wq", [D, HPG * DH], bf16, kind="ExternalInput")
    wk = nc.dram_tensor("wk", [D, HPG * DH], bf16, kind="ExternalInput")
    wv = nc.dram_tensor("wv", [D, HPG * DH], bf16, kind="ExternalInput")
    wo = nc.dram_tensor("wo", [HPG * DH, D], bf16, kind="ExternalInput")
    bq = nc.dram_tensor("bq", [HPG * DH], f32, kind="ExternalInput")
    bk = nc.dram_tensor("bk", [HPG * DH], f32, kind="ExternalInput")
    bv = nc.dram_tensor("bv", [1, HPG * DH], bf16, kind="ExternalInput")
    m01 = nc.dram_tensor("m01", [128, 128], bf16, kind="ExternalInput")
    ones_d = nc.dram_tensor("ones_d", [1, 128], bf16, kind="ExternalInput")
    ones2_d = nc.dram_tensor("ones2_d", [128, 128], f32, kind="ExternalInput")
    out = nc.dram_tensor("out", [S, D], bf16, kind="ExternalOutput")

    HD = HPG * DH  # 384

    from contextlib import ExitStack

    with tile.TileContext(nc) as tc:
        with ExitStack() as _ctx:
            _e = _ctx.enter_context
            _e(nc.allow_low_precision(reason="bf16 matmul pipeline"))
            wpool = _e(tc.tile_pool(name="weights", bufs=1))
            xtpool = _e(tc.tile_pool(name="xt", bufs=KT))
            qtpool = _e(tc.tile_pool(name="qt", bufs=NP * QC))
            ktpool = _e(tc.tile_pool(name="kt", bufs=NP * QC))
            vpool = _e(tc.tile_pool(name="v", bufs=ST))
            epool = _e(tc.tile_pool(name="e", bufs=5))
            zupool = _e(tc.tile_pool(name="zu", bufs=2))
            dnpool = _e(tc.tile_pool(name="dn", bufs=2))
            dnppool = _e(tc.tile_pool(name="dnp", bufs=2))
            zhpool = _e(tc.tile_pool(name="zh", bufs=2 * NP))
            opool = _e(tc.tile_pool(name="osb", bufs=3))
            stpool = _e(tc.tile_pool(name="st", bufs=2, space="PSUM"))
            pzpool = _e(tc.tile_pool(name="pz", bufs=2, space="PSUM"))
            miscpool = _e(tc.tile_pool(name="misc", bufs=2, space="PSUM"))

            # ---- small constants first (block nothing) ----
            ones_sb = wpool.tile([1, 128], bf16, tag="ones")
            nc.sync.dma_start(ones_sb[:], ones_d[:])
            ones2_sb = wpool.tile([128, 128], f32r, tag="ones2")
            nc.gpsimd.dma_start(ones2_sb[:], ones2_d[:].bitcast(f32r))
            bq_sb = wpool.tile([128, NP], f32, tag="bq")
            nc.sync.dma_start(bq_sb[:], bq[:].rearrange("(j p) -> p j", p=128))
            bk_sb = wpool.tile([128, NP], f32, tag="bk")
            nc.sync.dma_start(bk_sb[:], bk[:].rearrange("(j p) -> p j", p=128))
            bv_sb = wpool.tile([1, HD], bf16, tag="bv")
            nc.gpsimd.dma_start(bv_sb[:], bv[:])
            m01_sb = wpool.tile([128, 128], bf16, tag="m01")
            nc.gpsimd.dma_start(m01_sb[:], m01[:])

            # ---- PE warmup while input DMAs run (HAM un-throttle) ----
            wu = miscpool.tile([128, 128], f32, tag="misc", name="wu")
            for _ in range(20):
                nc.tensor.matmul(
                    wu[:], ones2_sb[:], ones2_sb[:], start=True, stop=True
                )

            # ---- weights and activations, split per k-tile block over two
            # DGE queues so the first projection matmuls start early ----
            wq_sb = wpool.tile([128, KT * HD], bf16, tag="wq")
            wk_sb = wpool.tile([128, KT * HD], bf16, tag="wk")
            wv_sb = wpool.tile([128, KT * HD], bf16, tag="wv")
            xt_sb = [xtpool.tile([128, S], bf16, tag="xt", name=f"xt{i}") for i in range(KT)]
            for a in range(KT):
                asl = slice(a * 128, (a + 1) * 128)
                nc.sync.dma_start(wq_sb[:, a * HD : (a + 1) * HD], wq[asl, :])
                nc.gpsimd.dma_start(wk_sb[:, a * HD : (a + 1) * HD], wk[asl, :])
            for c in range(QC):
                csl = slice(c * 512, (c + 1) * 512)
                for a in range(KT):
                    eng = nc.sync if a % 2 == 0 else nc.gpsimd
                    eng.dma_start(xt_sb[a][:, csl], xT[a * 128 : (a + 1) * 128, csl])
                if c == 0:
                    for a in range(KT):
                        eng = nc.sync if a % 2 == 1 else nc.gpsimd
                        eng.dma_start(
                            wv_sb[:, a * HD : (a + 1) * HD],
                            wv[a * 128 : (a + 1) * 128, :],
                        )
            wo_sb = wpool.tile([128, NP * D], bf16, tag="wo")
            nc.sync.dma_start(
                wo_sb[:].rearrange("p (j d) -> p j d", j=NP),
                wo[:].rearrange("(j p) d -> p j d", p=128),
            )

            # ---- persistent SBUF activation tiles ----
            qt_sb = [
                [qtpool.tile([128, 512], bf16, tag="qt", name=f"qt{i}_{cc}")
                 for cc in range(QC)]
                for i in range(NP)
            ]
            kt_sb = [
                [ktpool.tile([128, 512], bf16, tag="kt", name=f"kt{i}_{cc}")
                 for cc in range(QC)]
                for i in range(NP)
            ]
            v_sb = [vpool.tile([128, HPG * VS], bf16, tag="v", name=f"v{i}")
                    for i in range(ST)]

            def emit_proj_qk(c, j, which):
                csl = slice(c * 512, (c + 1) * 512)
                dst, w_sb, b_sb = (
                    (qt_sb[j][c], wq_sb, bq_sb)
                    if which == 0
                    else (kt_sb[j][c], wk_sb, bk_sb)
                )
                ps = miscpool.tile([128, 512], f32, tag="misc", name="ps")
                for a in range(KT):
                    nc.tensor.matmul(
                        ps[:],
                        w_sb[:, a * HD + j * 128 : a * HD + (j + 1) * 128],
                        xt_sb[a][:, csl],
                        start=(a == 0),
                        stop=(a == KT - 1),
                    )
                # PSUM->SBUF move + per-partition bias on DVE
                nc.vector.tensor_scalar_add(dst[:], ps[:], b_sb[:, j : j + 1])

            def emit_proj_v(st):
                vt = v_sb[st]
                pv = miscpool.tile([128, HD + HPG], f32, tag="misc", name="pv")
                for a in range(KT):
                    nc.tensor.matmul(
                        pv[:, 0:HD],
                        xt_sb[a][:, st * 128 : (st + 1) * 128],
                        wv_sb[:, a * HD : (a + 1) * HD],
                        start=(a == 0),
                        stop=False,
                    )
                nc.tensor.matmul(
                    pv[:, 0:HD], ones_sb[:, :], bv_sb[:, :],
                    start=False, stop=True,
                )
                # ones columns for the softmax denominator (z row 64)
                nc.tensor.matmul(
                    pv[:, HD : HD + HPG],
                    ones_sb[:, :], ones_sb[:, 0:HPG],
                    start=True, stop=True,
                )
                vtv = vt[:].rearrange("p (n c) -> p n c", n=HPG)
                nc.vector.tensor_copy(
                    vtv[:, :, 0:DH],
                    pv[:, 0:HD].rearrange("p (n c) -> p n c", n=HPG),
                )
                nc.vector.tensor_copy(
                    vtv[:, :, DH : DH + 1],
                    pv[:, HD : HD + HPG].rearrange("p (n c) -> p n c", n=HPG),
                )

            def proj_units(c):
                for j in range(NP):
                    yield lambda j=j: emit_proj_qk(c, j, 0)
                    yield lambda j=j: emit_proj_qk(c, j, 1)
                for st in range(4 * c, 4 * c + 4):
                    yield lambda st=st: emit_proj_v(st)

            # chunk 0 projections up front
            for u in proj_units(0):
                u()

            for c in range(QC):
                # filler: next chunk's projections, interleaved between
                # attention generations so the in-order PE queue always has
                # independent matmuls behind the exp-gated score matmuls.
                filler = iter(proj_units(c + 1)) if c + 1 < QC else iter(())

                # ---- attention for this q-chunk ----
                gens = _chunk_gens(c)
                zh = [None] * NP
                for j in range(NP):
                    pzA = pzpool.tile([65, 512], f32, tag="pz", name="pzA")
                    pzB = pzpool.tile([65, 512], f32, tag="pz", name="pzB")
                    def emit_pv(gen, etA, etB):
                        for (t, off, w, qoff, bnd) in gen:
                            last = t == 4 * c + 3
                            nc.tensor.matmul(
                                pzA[:, qoff : qoff + w],
                                v_sb[t][:, (2 * j) * VS : (2 * j) * VS + 65],
                                etA[:, off : off + w],
                                start=(t == 0), stop=last,
                            )
                            nc.tensor.matmul(
                                pzB[:, qoff : qoff + w],
                                v_sb[t][:, (2 * j + 1) * VS : (2 * j + 1) * VS + 65],
                                etB[:, off : off + w],
                                start=(t == 0), stop=last,
                            )

                    # software pipeline: PV trails scores/exp by one gen so
                    # the in-order PE queue never waits on the ACT exp; A/B
                    # staging tiles share two 2-bank slots so scores(g+1)
                    # overlap exp(g) instead of serializing on one buffer.
                    prev = None
                    for gen in gens:
                        gw = gen[-1][1] + gen[-1][2]  # packed width
                        ets = []
                        for half in range(2):
                            stg = stpool.tile([128, 1024], f32, tag="st", name="stg")
                            r0, r1 = (0, 64) if half == 0 else (64, 128)
                            for (t, off, w, qoff, bnd) in gen:
                                kc, ko = t // 4, (t % 4) * 128
                                nc.tensor.matmul(
                                    stg[:, off : off + w],
                                    kt_sb[j][kc][r0:r1, ko : ko + 128],
                                    qt_sb[j][c][r0:r1, qoff : qoff + w],
                                    start=True, stop=True,
                                )
                            et = epool.tile([128, 1024], bf16, tag="e", name="et")
                            nc.scalar.activation(
                                et[:, 0:gw], stg[:, 0:gw], EXP, scale=0.125
                            )
                            # causal 0/1 triangle on boundary tiles (post-exp)
                            for (t, off, w, qoff, bnd) in gen:
                                if bnd:
                                    nc.vector.tensor_mul(
                                        et[:, off : off + 128],
                                        et[:, off : off + 128],
                                        m01_sb[:],
                                    )
                            ets.append(et)
                        if prev is not None:
                            emit_pv(*prev)
                        prev = (gen, ets[0], ets[1])
                        for u in (next(filler, None),):
                            if u is not None:
                                u()
                    if prev is not None:
                        emit_pv(*prev)
                    # ---- z / denom staging ----
                    # denominators are row-shaped [1, 512]; a straight DVE
                    # reciprocal on rows is lane-starved (512 elems on one
                    # lane). Spread them over partitions with a small DMA,
                    # reciprocal [128, 8], spread back, then broadcast the
                    # recip row to all 128 partitions with a K=1 matmul.
                    zu = zupool.tile([128, 512], f32, tag="zu", name="zu")
                    dn = dnpool.tile([65, 1024], f32, tag="dn", name="dn")
                    nc.vector.tensor_copy(zu[0:64, :], pzA[0:64, :])
                    nc.vector.tensor_copy(zu[64:128, :], pzB[0:64, :])
                    nc.vector.tensor_copy(dn[64:65, 0:512], pzA[64:65, :])
                    nc.vector.tensor_copy(dn[64:65, 512:1024], pzB[64:65, :])
                    dnp = dnppool.tile([128, 8], f32, tag="dnp", name="dnp")
                    nc.gpsimd.dma_start(dnp[:], dn[64:65, :])
                    rp = dnppool.tile([128, 8], f32r, tag="rp", name="rp")
                    nc.vector.reciprocal(rp[:], dnp[:])
                    rrow = dnpool.tile([65, 1024], f32r, tag="rrow", name="rrow")
                    nc.gpsimd.dma_start(rrow[64:65, :], rp[:])
                    bcpA = miscpool.tile([128, 512], f32, tag="misc", name="bcpA")
                    nc.tensor.matmul(
                        bcpA[:], ones2_sb[64:65, 0:128], rrow[64:65, 0:512],
                        start=True, stop=True,
                    )
                    bcpB = miscpool.tile([128, 512], f32, tag="misc", name="bcpB")
                    nc.tensor.matmul(
                        bcpB[:], ones2_sb[64:65, 0:128], rrow[64:65, 512:1024],
                        start=True, stop=True,
                    )
                    zht = zhpool.tile([128, 512], bf16, tag="zh", name=f"zh{j}")
                    nc.vector.tensor_mul(zht[0:64, :], zu[0:64, :], bcpA[0:64, :])
                    nc.vector.tensor_mul(zht[64:128, :], zu[64:128, :], bcpB[64:128, :])
                    zh[j] = zht
                    for u in (next(filler, None),):
                        if u is not None:
                            u()

                # drain any remaining filler before W_O
                for u in filler:
                    u()

                # ---- W_O contraction for this q-chunk ----
                for qs in range(4):
                    row = c * 512 + qs * 128
                    for half in range(2):
                        po = miscpool.tile([128, 384], f32, tag="misc", name="po")
                        for j in range(NP):
                            nc.tensor.matmul(
                                po[:],
                                zh[j][:, qs * 128 : (qs + 1) * 128],
                                wo_sb[:, j * D + half * 384 : j * D + (half + 1) * 384],
                                start=(j == 0), stop=(j == NP - 1),
                            )
                        osb = opool.tile([128, 384], bf16, tag="osb", name="osb")
                        nc.vector.tensor_copy(osb[:], po[:])
                        nc.sync.dma_start(
                            out[row : row + 128, half * 384 : (half + 1) * 384],
                            osb[:],
                        )

    _split_drain_waits(nc, mybir)
    return nc


_nc_cache = None


def kernel(normalized_resid_pre, W_Q, W_K, W_V, W_O, b_Q, b_K, b_V, b_O):
    import ml_dtypes
    from concourse.bass_utils import run_bass_kernel_spmd

    global _nc_cache
    if _nc_cache is None:
        _nc_cache = build_program()
    nc = _nc_cache

    bf16 = ml_dtypes.bfloat16
    x = np.asarray(normalized_resid_pre, np.float32)

    # multiplicative causal mask for the diagonal 128x128 block:
    # keep (1.0) where k_local <= q_local, else 0.
    p = np.arange(128)[:, None]
    u = np.arange(128)[None, :]
    m01 = np.where(p <= u, 1.0, 0.0).astype(bf16)

    in_maps = []
    for c in range(N_CORES):
        b, g = c // G, c % G
        hs = slice(g * HPG, (g + 1) * HPG)
        in_maps.append(
            {
                "xT": np.ascontiguousarray(x[b].T).astype(bf16),
                "wq": np.ascontiguousarray(
                    np.asarray(W_Q)[hs].transpose(1, 0, 2).reshape(D, HPG * DH)
                ).astype(bf16),
                "wk": np.ascontiguousarray(
                    np.asarray(W_K)[hs].transpose(1, 0, 2).reshape(D, HPG * DH)
                ).astype(bf16),
                "wv": np.ascontiguousarray(
                    np.asarray(W_V)[hs].transpose(1, 0, 2).reshape(D, HPG * DH)
                ).astype(bf16),
                "wo": np.ascontiguousarray(
                    np.asarray(W_O)[hs].reshape(HPG * DH, D)
                ).astype(bf16),
                "bq": np.ascontiguousarray(
                    np.asarray(b_Q, np.float32)[hs].reshape(-1)
                ),
                "bk": np.ascontiguousarray(
                    np.asarray(b_K, np.float32)[hs].reshape(-1)
                ),
                "bv": np.ascontiguousarray(
                    np.asarray(b_V)[hs].reshape(1, -1)
                ).astype(bf16),
                "m01": m01,
                "ones_d": np.ones((1, 128), bf16),
                "ones2_d": np.ones((128, 128), np.float32),
            }
        )

    res = run_bass_kernel_spmd(nc, in_maps, core_ids=list(range(N_CORES)))
    out = np.zeros((B, S, D), np.float32)
    for c in range(N_CORES):
        out[c // G] += np.asarray(res.results[c]["out"], np.float32)
    out += np.asarray(b_O, np.float32)
    return out
